# revision 52
# baseline (speedup 1.0000x reference)
"""GCN layer on 8 TRN2 NeuronCores (Bass/Tile).

out = segment_sum(edge_vals[:,None] * (X @ W)[edge_col], edge_row, N)

Strategy (1D destination-node sharding):
  - Host: cast/transpose X -> XT bf16 (replicated to all 8 cores). Partition
    edges by destination shard (6250 rows/core), group by destination window
    (128 rows), and within each window split by source node-tile half
    (A: tiles 0..195, B: 196..390) so the int16 dma_gather indices stay in
    range AND so half-A gathers can start while phase 1 is still computing
    half B. Pad each (window, half) group to a multiple of 128 edges with
    zero-weight edges; tile counts are maxed across cores so all 8 cores run
    the identical (SPMD) program.
  - Device phase 1: XW = X @ W computed redundantly per core (TensorE bf16,
    fp32 PSUM), stored bf16 into a DRAM scratch with rows padded to 128 cols
    (256B - dma_gather's granularity). Rows are partition-major within each
    half (virtual row p*T_half + t_local) so stores are a few large DMAs and
    each half is a contiguous int16-addressable gather table.
  - Device phase 2: dma_gather (SWDGE, round-robined over 4 queues) fetches
    the XW rows for each 128-edge tile; VectorE builds S[e, r] = val[e] *
    (row_local[e] == r) via an iota compare; TensorE accumulates S^T @ rows
    into the window's PSUM [128, 64]. Two passes: pass A accumulates all
    windows' half-A edges into an SBUF partial-sum (so half-A gathers overlap
    phase 1's half-B compute), pass B adds the half-B edges and streams the
    finished windows out. No scatter races anywhere.
  - Host: concatenate the 8 output shards.
"""

from contextlib import ExitStack

import ml_dtypes
import numpy as np

import concourse.bacc as bacc
import concourse.bass as bass
import concourse.mybir as mybir
import concourse.tile as tile
from concourse._compat import get_trn_type
from concourse.bass_utils import run_bass_kernel_spmd

N_NODES = 50000
N_EDGES = 800000
F_IN = 256
F_OUT = 64
N_CORES = 8
SHARD = N_NODES // N_CORES  # 6250 destination rows per core
WIN = 128  # destination rows per PSUM accumulation window
BF16 = ml_dtypes.bfloat16
FP8 = ml_dtypes.float8_e4m3fn

NT = (N_NODES + 127) // 128  # 391 node tiles
TA = 192  # node tiles in half A (slab-aligned: 6 * (SLAB//128))
TB = NT - TA  # 195 node tiles in half B
ROWS_A = 128 * TA  # 25088 gather-table rows (int16-safe)
ROWS_B = 128 * TB  # 24960

# knobs
SLAB = 4096  # phase-1 node columns per XT slab DMA
GRP = 32  # phase-1 node tiles per staged XW store DMA (192 = 6*32)
CH = 12  # phase-2 edge tiles (of 128 edges) per dma_gather call
NQ = 4  # SWDGE queues used round-robin by gather chunks
GATH_BUFS = 6  # gather chunks in flight per ring tag (4 tags)
GB = 12  # phase-2 edge tiles per batched one-hot / rhs-scale (divides CH)
SC_EVERY = 3  # every SC_EVERY-th batch's rhs-scale runs on ScalarE (load balance)
SIM_MEMSET = False  # zero staging tiles (only needed to appease CoreSim)

# test.py pokes these for profiling
TRACE = False
LAST_RESULTS = None


def _install_ntff_hook():
    """The agent image's antenv lacks axon_hooks, so bass_utils' trace=True
    path can't find the NTFF hook. Recreate the module and register the
    ctypes-based hook exactly as trn_agent_boot would."""
    import sys
    import types

    try:
        import antenv.axon_hooks  # noqa: F401

        return True
    except ImportError:
        pass
    try:
        import antenv
        from trn_agent_boot.trn_boot import _ntff_profile_via_ctypes

        mod = types.ModuleType("antenv.axon_hooks")
        mod._hook = None

        def set_axon_ntff_profile_hook(h):
            mod._hook = h

        def get_axon_ntff_profile_hook():
            return mod._hook

        mod.set_axon_ntff_profile_hook = set_axon_ntff_profile_hook
        mod.get_axon_ntff_profile_hook = get_axon_ntff_profile_hook
        sys.modules["antenv.axon_hooks"] = mod
        antenv.axon_hooks = mod
        hook = _ntff_profile_via_ctypes("/opt/axon/libaxon_pjrt.so")
        if hook is not None:
            set_axon_ntff_profile_hook(hook)
        return hook is not None
    except Exception as e:  # profiling is best-effort
        print(f"ntff hook install failed: {e}")
        return False


def _wrap16(stream_i16, n_tiles):
    """Wrapped+replicated dma_gather index layout: stream position i lives at
    partition i%16 (replicated to all 8 16-partition groups), slot i//16."""
    n = n_tiles * 128
    w = np.zeros((128, n // 16), dtype=np.int16)
    s = np.zeros(n, dtype=np.int16)
    s[: len(stream_i16)] = stream_i16
    blk = s.reshape(n // 16, 16).T  # [16, n//16]
    for g in range(8):
        w[g * 16 : (g + 1) * 16, :] = blk
    return w


def _prep(X, W, edge_row, edge_col, edge_vals):
    """Host-side sharding/marshalling.

    Returns (in_maps, T_lo, T_hi): per-window tile counts for the A/B
    source halves (identical across cores -> shared SPMD program).
    """
    XT = np.ascontiguousarray(X.T).astype(BF16)  # [F_IN, N_NODES]
    Wb = np.ascontiguousarray(W).astype(BF16)  # [F_IN, F_OUT]
    iota = np.tile(np.arange(WIN, dtype=np.float32), (128, GB))  # [128, GB*WIN]
    # bf16 one-hot meta: rowloc/iota are small ints (exact); vals lose ~3
    # decimal digits which is within the bf16-XW error already accepted.

    n_win = (SHARD + WIN - 1) // WIN
    core = edge_row // SHARD
    percore = []
    cnt_lo = np.zeros((N_CORES, n_win), dtype=np.int64)
    cnt_hi = np.zeros((N_CORES, n_win), dtype=np.int64)
    for p in range(N_CORES):
        m = core == p
        r = edge_row[m].astype(np.int64) - p * SHARD
        c = edge_col[m].astype(np.int64)
        v = edge_vals[m].astype(np.float32)
        t = c // 128
        pp = c % 128
        hi = t >= TA
        q = np.where(hi, pp * TB + (t - TA), pp * TA + t)  # row within half
        w = r // WIN
        # order: (window, half) groups; ascending gather address within
        order = np.lexsort((q, hi, w))
        r, q, v, hi, w = r[order], q[order], v[order], hi[order], w[order]
        percore.append((r, q, v, hi, w))
        for wi in range(n_win):
            mw = w == wi
            cnt_lo[p, wi] = (mw & ~hi).sum()
            cnt_hi[p, wi] = (mw & hi).sum()

    T_lo = np.maximum(1, -(-cnt_lo.max(axis=0) // 128))
    T_hi = np.maximum(1, -(-cnt_hi.max(axis=0) // 128))
    J_lo, J_hi = int(T_lo.sum()), int(T_hi.sum())
    J = J_lo + J_hi
    lo_starts = np.concatenate([[0], np.cumsum(T_lo)])
    hi_starts = np.concatenate([[0], np.cumsum(T_hi)])

    in_maps = []
    for p in range(N_CORES):
        r, q, v, hi, w = percore[p]
        lo_q = np.zeros(J_lo * 128, dtype=np.int64)
        hi_q = np.zeros(J_hi * 128, dtype=np.int64)
        # meta in stream order: lo stream cols [0, J_lo), hi [J_lo, J)
        vals = np.zeros(J * 128, dtype=np.float32)
        rowloc = np.zeros(J * 128, dtype=np.float32)
        for wi in range(n_win):
            for is_hi, starts_h, qbuf, off in (
                (False, lo_starts, lo_q, 0),
                (True, hi_starts, hi_q, J_lo),
            ):
                mw = (w == wi) & (hi == is_hi)
                n = int(mw.sum())
                s0 = int(starts_h[wi]) * 128
                qbuf[s0 : s0 + n] = q[mw]
                mb = (off + int(starts_h[wi])) * 128
                vals[mb : mb + n] = v[mw]
                rowloc[mb : mb + n] = (r[mw] % WIN).astype(np.float32)
        meta = np.concatenate(
            [rowloc.reshape(J, 128).T, vals.reshape(J, 128).T, iota], axis=1
        ).astype(BF16)
        in_maps.append(
            {
                "xt": XT,
                "w": Wb,
                "cols_lo": _wrap16(lo_q.astype(np.int16), J_lo),
                "cols_hi": _wrap16(hi_q.astype(np.int16), J_hi),
                "meta": np.ascontiguousarray(meta),
            }
        )
    return in_maps, T_lo, T_hi


def _build_nc(T_lo, T_hi, n_nodes=N_NODES, f_in=F_IN, f_out=F_OUT, shard=SHARD):
    f32 = mybir.dt.float32
    bf16 = mybir.dt.bfloat16
    i16 = mybir.dt.int16
    n_win = len(T_lo)
    J_lo, J_hi = int(T_lo.sum()), int(T_hi.sum())
    J = J_lo + J_hi
    lo_starts = np.concatenate([[0], np.cumsum(T_lo)])
    hi_starts = np.concatenate([[0], np.cumsum(T_hi)])

    nc = bacc.Bacc(
        get_trn_type() or "TRN2",
        target_bir_lowering=False,
        dynamic_dma_scratch_size=32768,
        num_swdge_queues=NQ,
    )
    xt = nc.dram_tensor("xt", [f_in, n_nodes], bf16, kind="ExternalInput")
    w_in = nc.dram_tensor("w", [f_in, f_out], bf16, kind="ExternalInput")
    cols_lo = nc.dram_tensor("cols_lo", [128, J_lo * 8], i16, kind="ExternalInput")
    cols_hi = nc.dram_tensor("cols_hi", [128, J_hi * 8], i16, kind="ExternalInput")
    meta = nc.dram_tensor("meta", [128, 2 * J + GB * WIN], bf16, kind="ExternalInput")
    out = nc.dram_tensor("out", [shard, f_out], f32, kind="ExternalOutput")
    # XW scratch: half A rows [0, ROWS_A) hold virtual row p*TA + t (t<TA);
    # half B rows [ROWS_A, ROWS_A+ROWS_B) hold p*TB + (t-TA). Cols padded
    # 64->128 so each row is 256B (dma_gather granularity).
    xw = nc.dram_tensor("xw", [ROWS_A + ROWS_B, 128], bf16, kind="Internal")
    warmt = nc.dram_tensor("warmt", [128, 128], bf16, kind="Internal")

    n_kc = f_in // 128  # contraction chunks (2)
    CG = 8  # node tiles per PSUM bank (8*64 f32 = 2048B = bank); GRP = 4*CG
    ident = mybir.ActivationFunctionType.Identity

    with tile.TileContext(nc) as tc, ExitStack() as ctx:
        const = ctx.enter_context(tc.tile_pool(name="const", bufs=1))
        xt_pool = ctx.enter_context(tc.tile_pool(name="xtp", bufs=2))
        psum1 = ctx.enter_context(tc.tile_pool(name="psum1", bufs=3, space="PSUM"))
        xw_sb = ctx.enter_context(tc.tile_pool(name="xw_sb", bufs=2))
        gath = ctx.enter_context(tc.tile_pool(name="gath", bufs=GATH_BUFS))
        s_pool = ctx.enter_context(tc.tile_pool(name="s_pool", bufs=3))
        rhs_pool = ctx.enter_context(tc.tile_pool(name="rhs_pool", bufs=3))
        psum2 = ctx.enter_context(tc.tile_pool(name="psum2", bufs=5, space="PSUM"))
        out_sb = ctx.enter_context(tc.tile_pool(name="out_sb", bufs=4))

        # resident constants
        w_t = []
        for k in range(n_kc):
            wt = const.tile([128, f_out], bf16, tag=f"w{k}")
            nc.sync.dma_start(out=wt[:], in_=w_in[k * 128 : (k + 1) * 128, :])
            w_t.append(wt)
        meta_t = const.tile([128, 2 * J + GB * WIN], bf16, tag="meta")
        clo_t = const.tile([128, J_lo * 8], i16, tag="clo")
        chi_t = const.tile([128, J_hi * 8], i16, tag="chi")
        # half-A/B partial-sum accumulator for all windows
        acc = const.tile([128, n_win * f_out], f32, tag="acc")
        # fp32 copy of the per-edge scales (ScalarE activation needs FP32 AP)
        vals32 = const.tile([128, J], f32, tag="vals32")

        # Dummy gather: forces the GPSIMD dma_gather ucode library load (and
        # SWDGE queue init) to happen during phase 1 instead of serializing
        # ~11us in front of the first real gather. Reads the warmt scratch so
        # it has no phase-1 dependencies. Lands in the g0 ring (slot is later
        # overwritten and read by a real chunk, appeasing the BIR verifier).
        warm = gath.tile([128, CH, 128], bf16, tag="g0")
        warm_ix = const.tile([128, 8], i16, tag="warm_ix")
        nc.vector.memset(warm_ix[:], 0)
        nc.gpsimd.dma_gather(
            out_ap=warm[:, :1, :],
            in_ap=warmt[:, :],
            idxs_ap=warm_ix[:, :],
            num_idxs=128,
            num_idxs_reg=128,
            elem_size=128,
            single_packet=False,
            queue_num=0,
        )

        # ---- phase 1: xw = (X @ W) in bf16, half-major, 128-padded ----
        xwA = xw[0:ROWS_A, :].rearrange("(p t) f -> p (t f)", p=128)
        xwB = xw[ROWS_A : ROWS_A + ROWS_B, :].rearrange("(p t) f -> p (t f)", p=128)
        stg = None
        ps = None
        g0 = 0
        b0 = 0
        s0 = 0
        xts = []
        for nt_i in range(NT):
            n0 = nt_i * 128
            m = min(128, n_nodes - n0)
            if nt_i % (SLAB // 128) == 0:
                s0 = n0
                sl = min(SLAB, n_nodes - s0)
                xts = []
                for k in range(n_kc):
                    xtk = xt_pool.tile([128, SLAB], bf16, tag=f"xt{k}")
                    nc.sync.dma_start(
                        out=xtk[:, :sl],
                        in_=xt[k * 128 : (k + 1) * 128, s0 : s0 + sl],
                    )
                    xts.append(xtk)
            if nt_i == SLAB // 128:
                # deferred phase-2 constant loads: queued behind the first
                # slab pair so phase 1 starts ~15us earlier
                nc.sync.dma_start(out=meta_t[:], in_=meta[:, :])
                nc.sync.dma_start(out=clo_t[:], in_=cols_lo[:, :])
                nc.sync.dma_start(out=chi_t[:], in_=cols_hi[:, :])
                nc.vector.tensor_copy(out=vals32[:], in_=meta_t[:, J : 2 * J])
            if nt_i % GRP == 0:
                g0 = nt_i
                stg = xw_sb.tile([128, GRP * 128], bf16, tag="stg")
                if SIM_MEMSET:  # garbage bytes are never consumed on HW
                    nc.gpsimd.memset(stg[:], 0)
            if nt_i % CG == 0:
                b0 = nt_i
                ps = psum1.tile([128, CG, f_out], f32, tag="ps1")
            for k in range(n_kc):
                nc.tensor.matmul(
                    out=ps[:m, nt_i - b0, :],
                    lhsT=xts[k][:, n0 - s0 : n0 - s0 + m],
                    rhs=w_t[k][:],
                    start=(k == 0),
                    stop=(k == n_kc - 1),
                )
            if nt_i == NT - 1 or (nt_i + 1) % CG == 0:
                bn = nt_i + 1 - b0
                loc0 = b0 - g0
                stg_v = stg[:, loc0 * 128 : (loc0 + bn) * 128].rearrange(
                    "p (t f) -> p t f", f=128
                )
                nc.vector.tensor_copy(out=stg_v[:, :, 0:f_out], in_=ps[:, :bn, :])
            if nt_i == NT - 1 or (nt_i + 1) % GRP == 0:
                gn = nt_i + 1 - g0
                if g0 < TA:  # groups 0..6 -> half A (GRP*6 == TA)
                    dst = xwA[:, g0 * 128 : (g0 + gn) * 128]
                else:
                    dst = xwB[:, (g0 - TA) * 128 : (g0 - TA + gn) * 128]
                nc.sync.dma_start(out=dst, in_=stg[:, : gn * 128])

        # ---- phase 2: dma_gather + batched one-hot matmul segment-sum ----
        chunks = {}  # (is_hi, chunk_idx) -> gather tile
        batches = {}  # (is_hi, batch_idx) -> (S_b, rhs_b)
        issue_ctr = [0]  # round-robins gather chunks across SWDGE queues

        def ensure_chunk(is_hi, tile_idx):
            ci = tile_idx // CH
            key = (is_hi, ci)
            if key in chunks:
                return chunks[key]
            J_h = J_hi if is_hi else J_lo
            cols_t = chi_t if is_hi else clo_t
            base, rows_h = (ROWS_A, ROWS_B) if is_hi else (0, ROWS_A)
            cn = min(CH, J_h - ci * CH)
            g = gath.tile([128, CH, 128], bf16, tag=f"g{ci % 4}")
            nc.gpsimd.dma_gather(
                out_ap=g[:, :cn, :],
                in_ap=xw[base : base + rows_h, :],
                idxs_ap=cols_t[:, ci * CH * 8 : (ci * CH + cn) * 8],
                num_idxs=cn * 128,
                num_idxs_reg=cn * 128,
                elem_size=128,
                single_packet=False,
                queue_num=issue_ctr[0] % NQ,
            )
            issue_ctr[0] += 1
            chunks[key] = g
            return g

        def ensure_batch(is_hi, tile_idx):
            bi = tile_idx // GB
            key = (is_hi, bi)
            if key in batches:
                return batches[key]
            J_h = J_hi if is_hi else J_lo
            b0 = bi * GB
            bn = min(GB, J_h - b0)
            g = ensure_chunk(is_hi, b0)
            gs = b0 - (b0 // CH) * CH  # batch offset within its chunk
            mc = b0 + (J_lo if is_hi else 0)  # stream-ordered meta column
            S_b = s_pool.tile([128, GB, WIN], bf16, tag=f"S{bi % 2}")
            rhs_b = rhs_pool.tile([128, GB, f_out], bf16, tag=f"r{bi % 2}")
            nc.vector.tensor_tensor(
                out=S_b[:, :bn, :],
                in0=meta_t[:, 2 * J : 2 * J + bn * WIN].rearrange(
                    "p (b r) -> p b r", r=WIN
                ),
                in1=meta_t[:, mc : mc + bn].to_broadcast([128, bn, WIN]),
                op=mybir.AluOpType.is_equal,
            )
            if bi % SC_EVERY == SC_EVERY - 1:
                # offload this batch's per-edge scale to the idle ScalarE
                for j in range(bn):
                    nc.scalar.activation(
                        out=rhs_b[:, j, :],
                        in_=g[:, gs + j, 0:f_out],
                        func=ident,
                        scale=vals32[:, mc + j : mc + j + 1],
                    )
            else:
                nc.vector.tensor_tensor(
                    out=rhs_b[:, :bn, :],
                    in0=g[:, gs : gs + bn, 0:f_out],
                    in1=meta_t[:, J + mc : J + mc + bn].to_broadcast(
                        [128, bn, f_out]
                    ),
                    op=mybir.AluOpType.mult,
                )
            batches[key] = (S_b, rhs_b)
            return batches[key]

        for is_hi, starts_h, T_h in ((False, lo_starts, T_lo), (True, hi_starts, T_hi)):
            for w in range(n_win):
                cur_ps = psum2.tile([128, f_out], f32, tag="ps2")
                n_t = int(T_h[w])
                for k in range(n_t):
                    t_s = int(starts_h[w]) + k  # stream position
                    S_b, rhs_b = ensure_batch(is_hi, t_s)
                    sl = t_s % GB
                    nc.tensor.matmul(
                        out=cur_ps[:],
                        lhsT=S_b[:, sl : sl + 1, :],
                        rhs=rhs_b[:, sl : sl + 1, :],
                        start=(k == 0),
                        stop=(k == n_t - 1),
                    )
                if not is_hi:
                    # bank the half-A partial sums (ScalarE: keeps VectorE free)
                    nc.scalar.activation(
                        out=acc[:, w * f_out : (w + 1) * f_out],
                        in_=cur_ps[:],
                        func=ident,
                    )
                else:
                    rows = min(WIN, shard - w * WIN)
                    ot = out_sb.tile([128, f_out], f32, tag="ot")
                    nc.vector.tensor_tensor(
                        out=ot[:rows, :],
                        in0=cur_ps[:rows, :],
                        in1=acc[:rows, w * f_out : (w + 1) * f_out],
                        op=mybir.AluOpType.add,
                    )
                    nc.sync.dma_start(
                        out=out[w * WIN : w * WIN + rows, :], in_=ot[:rows, :]
                    )
    nc.compile()
    return nc


def kernel(X, W, edge_row, edge_col, edge_vals):
    global LAST_RESULTS
    X = np.asarray(X, dtype=np.float32)
    W = np.asarray(W, dtype=np.float32)
    edge_row = np.asarray(edge_row, dtype=np.int32)
    edge_col = np.asarray(edge_col, dtype=np.int32)
    edge_vals = np.asarray(edge_vals, dtype=np.float32)

    in_maps, T_lo, T_hi = _prep(X, W, edge_row, edge_col, edge_vals)
    nc = _build_nc(T_lo, T_hi)
    trace = TRACE and _install_ntff_hook()
    res = run_bass_kernel_spmd(
        nc, in_maps, core_ids=list(range(N_CORES)), trace=trace
    )
    LAST_RESULTS = res
    out = np.concatenate([res.results[p]["out"] for p in range(N_CORES)], axis=0)
    return out.astype(np.float32)
